# revision 1
# baseline (speedup 1.0000x reference)
"""Trainium2 Bass kernel for multi-head attention (nn_Attention_54984171323822).

Reference computation (fp32):
    qkv = x @ w_qkv.T + b_qkv            # [B, N, 3*1024]
    q, k, v -> 16 heads x 64
    attn = softmax(q k^T / 8) v          # per head
    out = attn_flat @ w_out.T + b_out    # [B, N, 1024]

Shapes: B=4, N=2048, HIDDEN=1024, 16 heads x 64.

Sharding (8 NeuronCores): DP=4 over batch x TP=2 over heads. Core c handles
batch c//2 and heads (c%2)*8..(c%2)*8+8. No device collectives: each core
emits a partial output-projection [2048, 1024]; the host sums the TP pairs
and adds b_out (linear, so it commutes).

Per-core device program (all layouts transposed, i.e. [feature, token], so
no on-chip transposes are ever needed):
  1. qkT = w_qkv_local @ x^T           (e on partitions)   + bias
     V   = x @ w_v_local^T             (tokens on partitions), stored
           interleaved as V'' = per-head [V_h | ones] blocks of 65 cols.
  2. per head: S^T[k,q] = K_h @ Q_h^T  (K=64 matmuls)
     E^T = exp(S^T / 8)                (ScalarE, straight from PSUM)
     oT'[j,q] = sum_k V''_h[k,j] E^T[k,q]   -> rows 0..63 = unnormalized o^T,
                                               row 64 = softmax denominator.
     attnT_h = oT' * recip(denom)      (denominator broadcast via GpSimd)
  3. out = attnT^T @ w_outT            (tokens on partitions again)

The no-max-subtraction softmax is safe here: logits are ~N(0, 0.5^2) after
the 1/8 scale, so exp() stays in (1e-3, ~20).
"""

import sys

sys.path.insert(0, "/opt/trn_rl_repo")

import numpy as np
import ml_dtypes

import concourse.bass as bass
import concourse.bacc as bacc
import concourse.tile as tile
from concourse import mybir
from concourse import bass_utils

N_CORES = 8
B = 4
N = 2048
HIDDEN = 1024
N_HEADS = 16
HEAD_DIM = 64
HPC = N_HEADS // 2          # heads per core (TP=2)
EC = HPC * HEAD_DIM         # 512 attention dims per core
TC = N // 128               # 16 token chunks
DC = HIDDEN // 128          # 8 hidden chunks
SCALE = HEAD_DIM ** -0.5

BF16 = mybir.dt.bfloat16
F32 = mybir.dt.float32
NP_BF16 = ml_dtypes.bfloat16


def _build_kernel_body(nc, tc_ctx, ios, dbg=None):
    import contextlib

    xT, wqkvT, bias_qk, bias_v, w_outT, out = ios
    tc = tc_ctx
    ctx = contextlib.ExitStack()
    with ctx:
        const = ctx.enter_context(tc.tile_pool(name="const", bufs=1))
        work = ctx.enter_context(tc.tile_pool(name="work", bufs=3))
        etp = ctx.enter_context(tc.tile_pool(name="etp", bufs=4))
        small = ctx.enter_context(tc.tile_pool(name="small", bufs=2))
        accp = ctx.enter_context(tc.tile_pool(name="accp", bufs=4, space="PSUM"))
        stp = ctx.enter_context(tc.tile_pool(name="stp", bufs=2, space="PSUM"))

        # ---- resident SBUF tensors ----
        # chunked loads so the first QKV matmuls start after ~1 chunk, not
        # after the full 7 MB of weights+activations land
        xT_src = xT.ap().rearrange("(c p) t -> c p t", p=128)
        wq_src = wqkvT.ap().rearrange("(c p) e -> c p e", p=128)
        xT_c = []
        wq_c = []
        for dc in range(DC):
            wt = const.tile([128, 3 * EC], BF16, name=f"wq{dc}", tag=f"wq{dc}")
            nc.scalar.dma_start(out=wt[:], in_=wq_src[dc])
            wq_c.append(wt)
            xt = const.tile([128, N], BF16, name=f"xc{dc}", tag=f"xc{dc}")
            nc.sync.dma_start(out=xt[:], in_=xT_src[dc])
            xT_c.append(xt)
        wo_sb = const.tile([128, EC // 128, HIDDEN], BF16, name="wo_sb", tag="wo_sb")
        nc.sync.dma_start(out=wo_sb[:], in_=w_outT.ap().rearrange("(c p) e -> p c e", p=128))
        bqk_sb = const.tile([128, 8], F32, name="bqk_sb", tag="bqk_sb")
        nc.sync.dma_start(out=bqk_sb[:], in_=bias_qk.ap())
        # bias_v broadcast to all partitions ([1, 520] dram, partition step 0)
        bv_sb = const.tile([128, HPC * 65], BF16, name="bv_sb", tag="bv_sb")
        bv_ap = bias_v.ap()
        bv_bcast = bass.AP(tensor=bv_ap.tensor, offset=bv_ap.offset,
                           ap=[[0, 128], [1, HPC * 65]])
        nc.gpsimd.dma_start(out=bv_sb[:], in_=bv_bcast)

        qkT = const.tile([128, 2 * EC // 128, N], BF16, name="qkT", tag="qkT")   # [128, 8, 2048]
        vpp = const.tile([128, TC, HPC * 65], BF16, name="vpp", tag="vpp")       # V'' tiles
        attnT_c = [const.tile([128, N], BF16, name=f"attnT{i}", tag=f"attnT{i}")
                   for i in range(EC // 128)]

        ones64 = const.tile([128, 64], F32, name="ones64", tag="ones64")
        nc.vector.memset(ones64[:], 1.0)

        # ones columns of V'' (col h*65+64 of every token chunk)
        ones_ap = vpp[:].rearrange("p t (h u) -> p t h u", u=65)[:, :, :, 64:65]
        nc.vector.memset(ones_ap, 1.0)

        # ---- phase 1: qkT = w_qk @ x^T + b (e on partitions) ----
        for ec in range(2 * EC // 128):                      # 8 chunks (q then k)
            for ti in range(4):
                ps = accp.tile([128, 512], F32, name="acc", tag="acc")
                for dc in range(DC):
                    for eh in range(2):   # col-group pair (output rows 0-63/64-127)
                        nc.tensor.matmul(
                            ps[eh * 64:(eh + 1) * 64, :],
                            wq_c[dc][:, ec * 128 + eh * 64:ec * 128 + (eh + 1) * 64],
                            xT_c[dc][:, ti * 512:(ti + 1) * 512],
                            start=(dc == 0), stop=(dc == DC - 1),
                        )
                nc.vector.tensor_scalar_add(
                    qkT[:, ec, ti * 512:(ti + 1) * 512], ps[:],
                    bqk_sb[:, ec:ec + 1],
                )

        # ---- phase 1b: V (tokens on partitions) + bias, into V'' layout ----
        for ti in range(TC):
            ps = accp.tile([128, 512], F32, name="acc", tag="acc")
            for dc in range(DC):
                for th in range(2):   # col-group pair
                    nc.tensor.matmul(
                        ps[th * 64:(th + 1) * 64, :],
                        xT_c[dc][:, ti * 128 + th * 64:ti * 128 + (th + 1) * 64],
                        wq_c[dc][:, 2 * EC:3 * EC],
                        start=(dc == 0), stop=(dc == DC - 1),
                    )
            v_out = vpp[:, ti].rearrange("p (h u) -> p h u", u=65)[:, :, 0:64]
            v_in = ps[:].rearrange("p (h u) -> p h u", u=64)
            v_bias = bv_sb[:].rearrange("p (h u) -> p h u", u=65)[:, :, 0:64]
            nc.vector.tensor_tensor(out=v_out, in0=v_in, in1=v_bias,
                                    op=mybir.AluOpType.add)

        # ---- phase 2: attention per head ----
        # S^T matmuls are issued as M=64 col-group pairs (output partitions
        # 0..63 / 64..127 of the same psum bank): disjoint column groups can
        # run concurrently in the PE array and hide each other's LDWEIGHTS.
        for h in range(HPC):
            qb = (h % 2) * 64            # partition base of this head's q/k rows
            qc = h // 2                  # q chunk
            kch = 4 + h // 2             # k chunk
            ac = h // 2                  # attnT chunk

            pv = [accp.tile([65, 512], F32, name="accpv", tag="acc") for _ in range(4)]

            def emit_pv(et_prev, kc_prev):
                for qi in range(4):
                    nc.tensor.matmul(
                        pv[qi][:],
                        vpp[:, kc_prev, h * 65:(h + 1) * 65],
                        et_prev[:, qi * 512:(qi + 1) * 512],
                        start=(kc_prev == 0), stop=(kc_prev == TC - 1),
                    )

            # PV lags the score/exp stream by one kc: the in-order PE queue
            # never waits on ScalarE for the just-issued exps.
            PV_LAG = 1
            pending = []
            for kc in range(TC):
                et = etp.tile([128, N], BF16, name="et", tag="et")
                for half in range(2):
                    st = stp.tile([128, 1024], F32, name="st", tag="st")
                    for j in range(2):
                        qs = half * 1024 + j * 512
                        for kh in range(2):   # col-group pair: k rows 0-63 / 64-127
                            nc.tensor.matmul(
                                st[kh * 64:(kh + 1) * 64, j * 512:(j + 1) * 512],
                                qkT[qb:qb + 64, kch,
                                    kc * 128 + kh * 64:kc * 128 + (kh + 1) * 64],
                                qkT[qb:qb + 64, qc, qs:qs + 512],
                                start=True, stop=True,
                            )
                    nc.scalar.activation(
                        out=et[:, half * 1024:(half + 1) * 1024], in_=st[:],
                        func=mybir.ActivationFunctionType.Exp, scale=SCALE,
                    )
                pending.append((et, kc))
                if len(pending) > PV_LAG:
                    emit_pv(*pending.pop(0))
            for p in pending:
                emit_pv(*p)

            # normalize: attnT_h[d, q] = oT'[d, q] * recip(oT'[64, q])
            # (recip over the whole [65, 512] tile: the custom-DVE op breaks on
            #  single-row base-64 APs; rows 0..63 are computed and discarded.
            #  HW partition_broadcast always reads partition 0, so DMA the
            #  recip row down to partition 0 first.)
            rec = small.tile([65, 4, 512], F32, name="rec", tag="rec", bufs=2)
            for qi in range(4):
                nc.vector.reciprocal_approx_fast(
                    out=rec[0:65, qi, :], in_=pv[qi][:])
            if h == HPC - 1:
                # last head: broadcast the recip row via a K=1 PE matmul into
                # a free st-pool psum slot, then copy to SBUF -- ~2us chain
                # instead of the ~8us DMA+gpsimd path, so the projection
                # (gated on this head's pv slot release) starts sooner.
                bcp = stp.tile([64, 1024], F32, name="bcp", tag="st")
                bcp2 = stp.tile([64, 1024], F32, name="bcp2", tag="st")
                bc = small.tile([64, 4, 512], F32, name="bc", tag="bc", bufs=2)
                for qi in range(4):
                    tgt = (bcp if qi < 2 else bcp2)
                    nc.tensor.matmul(
                        tgt[:, (qi % 2) * 512:(qi % 2 + 1) * 512],
                        ones64[64:65, :], rec[64:65, qi, :],
                        start=True, stop=True)
                    nc.vector.tensor_copy(
                        bc[:, qi, :], tgt[:, (qi % 2) * 512:(qi % 2 + 1) * 512])
            else:
                den0 = small.tile([1, 4, 512], F32, name="den0", tag="den0", bufs=1)
                nc.sync.dma_start(out=den0[:], in_=rec[64:65, :, :])
                bc = small.tile([64, 4, 512], F32, name="bc", tag="bc", bufs=2)
                nc.gpsimd.partition_broadcast(bc[:], den0[:], channels=64)
            if h % 2 == 0:
                for qi in range(4):
                    nc.vector.tensor_tensor(
                        out=attnT_c[ac][0:64, qi * 512:(qi + 1) * 512],
                        in0=pv[qi][0:64, :], in1=bc[0:64, qi, :],
                        op=mybir.AluOpType.mult)
            else:
                todd = small.tile([64, N], BF16, name="todd", tag="todd", bufs=2)
                for qi in range(4):
                    nc.vector.tensor_tensor(
                        out=todd[:, qi * 512:(qi + 1) * 512],
                        in0=pv[qi][0:64, :], in1=bc[0:64, qi, :],
                        op=mybir.AluOpType.mult)
                nc.sync.dma_start(out=attnT_c[ac][64:128, :], in_=todd[:])

        if dbg is not None:
            for nm, t in (("qkT", qkT), ("vpp", vpp)):
                if nm in dbg:
                    nc.sync.dma_start(out=dbg[nm].ap(), in_=t[:])

        # ---- phase 3: out = attnT^T @ w_outT ----
        out3 = out.ap().rearrange("(t p) e -> t p e", p=128)
        for ti in range(TC):
            osb = work.tile([128, HIDDEN], F32, name="osb", tag="osb")
            for e5 in range(2):
                po = accp.tile([128, 512], F32, name="acc", tag="acc")
                for acx in range(EC // 128):
                    for th in range(2):   # col-group pair
                        nc.tensor.matmul(
                            po[th * 64:(th + 1) * 64, :],
                            attnT_c[acx][:, ti * 128 + th * 64:ti * 128 + (th + 1) * 64],
                            wo_sb[:, acx, e5 * 512:(e5 + 1) * 512],
                            start=(acx == 0), stop=(acx == EC // 128 - 1),
                        )
                nc.vector.tensor_copy(osb[:, e5 * 512:(e5 + 1) * 512], po[:])
            nc.sync.dma_start(out=out3[ti], in_=osb[:])


def build_nc(debug_dump=False, num_devices=N_CORES):
    nc = bacc.Bacc("TRN2", target_bir_lowering=False, debug=False,
                   num_devices=num_devices)
    xT = nc.dram_tensor("xT", [HIDDEN, N], BF16, kind="ExternalInput")
    wqkvT = nc.dram_tensor("wqkvT", [HIDDEN, 3 * EC], BF16, kind="ExternalInput")
    bias_qk = nc.dram_tensor("bias_qk", [128, 8], F32, kind="ExternalInput")
    bias_v = nc.dram_tensor("bias_v", [1, HPC * 65], BF16, kind="ExternalInput")
    w_outT = nc.dram_tensor("w_outT", [EC, HIDDEN], BF16, kind="ExternalInput")
    out = nc.dram_tensor("out", [N, HIDDEN], F32, kind="ExternalOutput")
    dbg = None
    if debug_dump:
        dbg = {
            "qkT": nc.dram_tensor("dbg_qkT", [128, 8, N], BF16, kind="ExternalOutput"),
            "vpp": nc.dram_tensor("dbg_vpp", [128, TC, HPC * 65], BF16, kind="ExternalOutput"),
            "attnT": nc.dram_tensor("dbg_attnT", [128, 4, N], BF16, kind="ExternalOutput"),
        }
    with tile.TileContext(nc) as tc:
        _build_kernel_body(nc, tc, (xT, wqkvT, bias_qk, bias_v, w_outT, out), dbg=dbg)
    nc.compile()
    return nc


def make_in_maps(x, w_qkv, b_qkv, w_out):
    """Shard the full inputs into 8 per-core input maps."""
    in_maps = []
    for c in range(N_CORES):
        b = c // 2
        tp = c % 2
        sl = slice(tp * EC, (tp + 1) * EC)
        xT_c = np.ascontiguousarray(x[b].T).astype(NP_BF16)
        wq = w_qkv[sl, :]
        wk = w_qkv[HIDDEN + tp * EC: HIDDEN + (tp + 1) * EC, :]
        wv = w_qkv[2 * HIDDEN + tp * EC: 2 * HIDDEN + (tp + 1) * EC, :]
        wqkvT_c = np.concatenate([wq, wk, wv], axis=0).T.astype(NP_BF16)
        wqkvT_c = np.ascontiguousarray(wqkvT_c)
        bq = b_qkv[tp * EC:(tp + 1) * EC]
        bk = b_qkv[HIDDEN + tp * EC: HIDDEN + (tp + 1) * EC]
        bv = b_qkv[2 * HIDDEN + tp * EC: 2 * HIDDEN + (tp + 1) * EC]
        bias_qk_c = np.concatenate([bq, bk]).reshape(8, 128).T.astype(np.float32)
        bias_qk_c = np.ascontiguousarray(bias_qk_c)
        bias_v_c = np.zeros((1, HPC * 65), np.float32)
        bias_v_c.reshape(HPC, 65)[:, :64] = bv.reshape(HPC, 64)
        bias_v_c = bias_v_c.astype(NP_BF16)
        w_outT_c = np.ascontiguousarray(w_out[:, sl].T).astype(NP_BF16)
        in_maps.append({
            "xT": xT_c,
            "wqkvT": wqkvT_c,
            "bias_qk": bias_qk_c,
            "bias_v": bias_v_c,
            "w_outT": w_outT_c,
        })
    return in_maps


def combine_outputs(results, b_out):
    """results: list of 8 per-core {'out': [N, HIDDEN]} -> full [B, N, HIDDEN]."""
    out = np.empty((B, N, HIDDEN), np.float32)
    for b in range(B):
        out[b] = results[2 * b]["out"] + results[2 * b + 1]["out"]
        out[b] += b_out[None, :].astype(np.float32)
    return out


_NC = None


def _get_nc():
    global _NC
    if _NC is None:
        _NC = build_nc()
    return _NC


def kernel(x, w_qkv, b_qkv, w_out, b_out):
    x = np.asarray(x, np.float32)
    w_qkv = np.asarray(w_qkv, np.float32)
    b_qkv = np.asarray(b_qkv, np.float32)
    w_out = np.asarray(w_out, np.float32)
    b_out = np.asarray(b_out, np.float32)
    nc = _get_nc()
    in_maps = make_in_maps(x, w_qkv, b_qkv, w_out)
    res = bass_utils.run_bass_kernel_spmd(nc, in_maps, core_ids=list(range(N_CORES)))
    return combine_outputs(res.results, b_out)

